# revision 15
# baseline (speedup 1.0000x reference)
"""Trainium2 Bass kernel for a 2-layer LSTM (T=1024, B=4096, IN=1, H=16) + linear head.

Strategy: pure data parallel over batch (B=4096 -> 512 per core on 8 cores).
Within a core, the two LSTM layers are fused into a single recurrence with
layer 1 running one timestep behind layer 0, so each step needs ONE gates
matmul (K=33 = [h1(16); h0(16); x(1)], M=128 = all 8 gate blocks of both
layers), one sigmoid + one tanh activation over the PSUM gate rows and four
vector ops for the cell/hidden updates. The per-core batch is further split
into GROUPS independent interleaved chains so the engines overlap across
groups. States stay resident in SBUF; x is staged in SBUF up front; the fc
head is a [16]-dot applied on host to the exported y1 (h1 history).

The gates matmul runs as float32r (1 cycle/row for N>=256, vs 4 for fp32).
"""
import numpy as np

import concourse.bass as bass
import concourse.bacc as bacc
import concourse.tile as tile
from concourse import mybir
from concourse.bass_utils import run_bass_kernel_spmd

T, B, H, NCORES = 1024, 4096, 16, 8
BL = B // NCORES  # 512 batch elements per core
K = 2 * H + 1     # 33 rows: h1, h0, x
M = 8 * H         # 128 gate rows
GROUPS = 2        # independent interleaved chains per core

F32 = mybir.dt.float32
F32R = mybir.dt.float32r
AF = mybir.ActivationFunctionType

# PyTorch gate order inside 4H weight matrices: i, f, g, o
_GS = {'i': slice(0, H), 'f': slice(H, 2 * H), 'g': slice(2 * H, 3 * H), 'o': slice(3 * H, 4 * H)}
# Output (M) column order of the combined gates matmul: 16 cols per entry.
# Layer 1 first in each pair so the step-0 pipeline-bubble memsets target
# partition base 0 (compute ops may only start at partitions {0,32,64,96}).
# sigmoid rows 0:96 = [i1,i0,f1,f0,o1,o0], tanh rows 96:128 = [g1,g0]
_COL_ORDER = [('i', 1), ('i', 0), ('f', 1), ('f', 0), ('o', 1), ('o', 0), ('g', 1), ('g', 0)]


def _pack_weights(W_ih0, W_hh0, b_ih0, b_hh0, W_ih1, W_hh1, b_ih1, b_hh1):
    Wcomb = np.zeros((K, M), np.float32)
    bcomb = np.zeros((M, 1), np.float32)
    for ci, (gate, layer) in enumerate(_COL_ORDER):
        cols = slice(ci * H, (ci + 1) * H)
        gs = _GS[gate]
        if layer == 0:
            # state rows: 0:16 h1, 16:32 h0, 32 x
            Wcomb[2 * H, cols] = W_ih0[gs, 0]
            Wcomb[H:2 * H, cols] = W_hh0[gs, :].T
            bcomb[cols, 0] = b_ih0[gs] + b_hh0[gs]
        else:
            Wcomb[H:2 * H, cols] = W_ih1[gs, :].T
            Wcomb[0:H, cols] = W_hh1[gs, :].T
            bcomb[cols, 0] = b_ih1[gs] + b_hh1[gs]
    # g rows are evaluated as sigmoid(2g) (tanh(g) = 2*sigmoid(2g) - 1), so
    # fold the factor 2 into the last 32 gate columns
    Wcomb[:, 6 * H:8 * H] *= 2.0
    bcomb[6 * H:8 * H, 0] *= 2.0
    return Wcomb, bcomb


def _build(t_steps=T, bl=BL, groups=GROUPS):
    gb = bl // groups  # batch per group
    nc = bacc.Bacc()
    x_d = nc.declare_dram_parameter("x", [t_steps, bl], F32, isOutput=False)
    w_d = nc.declare_dram_parameter("wcomb", [K, M], F32, isOutput=False)
    b_d = nc.declare_dram_parameter("bcomb", [M, 1], F32, isOutput=False)
    y1_d = nc.declare_dram_parameter("y1", [t_steps, H, bl], F32, isOutput=True)
    hT_d = nc.declare_dram_parameter("hT", [2 * H, bl], F32, isOutput=True)
    cT_d = nc.declare_dram_parameter("cT", [2 * H, bl], F32, isOutput=True)

    P = min(t_steps, 128)           # x staging partitions
    NCH = (t_steps + P - 1) // P    # x staging chunks
    assert P * NCH == t_steps

    with tile.TileContext(nc) as tc:
        with (
            tc.tile_pool(name="singles", bufs=1) as singles,
            tc.tile_pool(name="psg", bufs=2, space="PSUM") as psg_pool,
            tc.tile_pool(name="psu", bufs=2, space="PSUM") as psu_pool,
            tc.tile_pool(name="acts", bufs=3) as acts_pool,
            tc.tile_pool(name="cells", bufs=3) as cells_pool,
        ):
            w_sb = singles.tile([K, M], F32)
            nc.sync.dma_start(out=w_sb[:, :].bitcast(F32R), in_=w_d[:, :].bitcast(F32R))
            b_sb = singles.tile([M, 1], F32)
            nc.sync.dma_start(out=b_sb[:, :], in_=b_d[:, :])

            x_all = singles.tile([P, NCH, bl], F32)
            nc.sync.dma_start(out=x_all[:, :, :], in_=x_d.rearrange("(n p) b -> p n b", p=P))

            zb = singles.tile([4 * H, 1], F32)
            nc.vector.memset(zb[:, :], 0.0)

            # Per-group persistent state: rows 0:16 h1, 16:32 h0, row 32 x.
            # c lives at rows 32:64 of a 64-row tile so every TensorTensor's
            # two inputs share a start partition (walrus requires equal SBUF
            # input start partitions, and starts must be in {0,32,64,96}).
            st = [[singles.tile([K, gb], F32, name=f"state{g}_{i}") for i in range(2)]
                  for g in range(groups)]
            cs = [[singles.tile([4 * H, gb], F32, name=f"cell{g}_{i}") for i in range(2)]
                  for g in range(groups)]
            for g in range(groups):
                gsl = slice(g * gb, (g + 1) * gb)
                nc.sync.dma_start(out=st[g][0][2 * H:2 * H + 1, :].bitcast(F32R),
                                  in_=x_all[0:1, 0, gsl].bitcast(F32R))
                nc.vector.memset(st[g][0][0:2 * H, :], 0.0)
                nc.vector.memset(cs[g][0][2 * H:4 * H, :], 0.0)

            for t in range(t_steps + 1):
                sigs, gts, psums = [], [], []
                for g in range(groups):
                    S = st[g][t % 2]
                    psum_g = psg_pool.tile([M, gb], F32, tag=f"psg{g}", name=f"psg{g}")
                    nc.tensor.matmul(psum_g[:, :], w_sb[:, :].bitcast(F32R),
                                     S[:, :].bitcast(F32R), start=True, stop=True)
                    sig = psu_pool.tile([8 * H, gb], F32, tag=f"sig{g}", name=f"sig{g}")
                    nc.scalar.activation(sig[:, :], psum_g[:, :], AF.Sigmoid,
                                         bias=b_sb[:, :], scale=1.0)
                    gt = acts_pool.tile([2 * H, gb], F32, tag=f"gt{g}", name=f"gt{g}")
                    nc.vector.tensor_scalar(gt[:, :], sig[6 * H:8 * H, :], 2.0, 1.0,
                                            mybir.AluOpType.mult, mybir.AluOpType.subtract)
                    sigs.append(sig)
                    gts.append(gt)

                for g in range(groups):
                    sig, gt = sigs[g], gts[g]
                    cc = cs[g][t % 2]
                    cn = cs[g][(t + 1) % 2]
                    fcm = cells_pool.tile([4 * H, gb], F32, tag=f"fcm{g}", name=f"fcm{g}")
                    nc.vector.tensor_mul(fcm[2 * H:4 * H, :], sig[2 * H:4 * H, :], cc[2 * H:4 * H, :])
                    mt = cells_pool.tile([4 * H, gb], F32, tag=f"mt{g}", name=f"mt{g}")
                    nc.vector.tensor_mul(mt[2 * H:4 * H, :], sig[0:2 * H, :], gt[:, :])
                    nc.vector.tensor_add(cn[2 * H:4 * H, :], fcm[2 * H:4 * H, :], mt[2 * H:4 * H, :])

                for g in range(groups):
                    gsl = slice(g * gb, (g + 1) * gb)
                    sig = sigs[g]
                    Sn = st[g][(t + 1) % 2]
                    cn = cs[g][(t + 1) % 2]
                    tcl = cells_pool.tile([6 * H, gb], F32, tag=f"tcl{g}", name=f"tcl{g}")
                    nc.scalar.activation(tcl[4 * H:6 * H, :], cn[2 * H:4 * H, :], AF.Tanh,
                                         bias=zb[2 * H:4 * H, :])
                    nc.vector.tensor_mul(Sn[0:2 * H, :].bitcast(F32R), sig[4 * H:6 * H, :], tcl[4 * H:6 * H, :])

                    if t == 0:
                        # layer1 pipeline bubble: zero the garbage h1/c1 of step 0
                        nc.vector.memset(Sn[0:H, :], 0.0)
                        nc.vector.memset(cn[2 * H:3 * H, :], 0.0)

                    if t >= 1:
                        # export h1_{t-1} (rows 0:16 of Sn); fc head applied on host
                        nc.sync.dma_start(out=y1_d[t - 1, :, gsl], in_=Sn[0:H, :])

                    # prefetch x_{t+1} into the next state tile's x row
                    if t + 1 <= t_steps - 1:
                        tn = t + 1
                        nc.sync.dma_start(out=Sn[2 * H:2 * H + 1, :].bitcast(F32R),
                                          in_=x_all[tn % P:tn % P + 1, tn // P, gsl].bitcast(F32R))

            # final hidden states (hT_d rows 0:16 = h0, 16:32 = h1):
            #   h0_{T-1} lives in st[.][T%2] rows 16:32 (written at step T-1)
            #   h1_{T-1} lives in st[.][(T+1)%2] rows 0:16 (written at step T)
            for g in range(groups):
                gsl = slice(g * gb, (g + 1) * gb)
                nc.sync.dma_start(out=hT_d[0:H, gsl], in_=st[g][t_steps % 2][H:2 * H, :])
                nc.sync.dma_start(out=hT_d[H:2 * H, gsl], in_=st[g][(t_steps + 1) % 2][0:H, :])
                nc.sync.dma_start(out=cT_d[0:H, gsl], in_=cs[g][t_steps % 2][3 * H:4 * H, :])
                nc.sync.dma_start(out=cT_d[H:2 * H, gsl], in_=cs[g][(t_steps + 1) % 2][2 * H:3 * H, :])

    nc.compile()
    return nc


_NC_CACHE = {}


def _get_nc(t_steps=T, bl=BL):
    key = (t_steps, bl)
    if key not in _NC_CACHE:
        _NC_CACHE[key] = _build(t_steps, bl)
    return _NC_CACHE[key]


def kernel(x, W_ih0, W_hh0, b_ih0, b_hh0, W_ih1, W_hh1, b_ih1, b_hh1, W_fc, b_fc):
    x = np.asarray(x, np.float32)
    Wcomb, bcomb = _pack_weights(
        np.asarray(W_ih0, np.float32), np.asarray(W_hh0, np.float32),
        np.asarray(b_ih0, np.float32), np.asarray(b_hh0, np.float32),
        np.asarray(W_ih1, np.float32), np.asarray(W_hh1, np.float32),
        np.asarray(b_ih1, np.float32), np.asarray(b_hh1, np.float32))

    nc = _get_nc()
    xs = x[:, :, 0]  # [T, B]
    in_maps = []
    for i in range(NCORES):
        sl = slice(i * BL, (i + 1) * BL)
        in_maps.append({
            "x": np.ascontiguousarray(xs[:, sl]),
            "wcomb": Wcomb, "bcomb": bcomb,
        })
    res = run_bass_kernel_spmd(nc, in_maps, list(range(NCORES))).results

    wfc = np.asarray(W_fc, np.float32)[0]  # [H]
    out = np.empty((T, B, 1), np.float32)
    h = np.empty((2, B, H), np.float32)
    c = np.empty((2, B, H), np.float32)
    for i in range(NCORES):
        sl = slice(i * BL, (i + 1) * BL)
        # y1: [T, H, BL] -> out = sum_h y1*wfc
        out[:, sl, 0] = np.einsum('thb,h->tb', res[i]["y1"], wfc, optimize=True)
        h[0, sl, :] = res[i]["hT"][0:H].T
        h[1, sl, :] = res[i]["hT"][H:2 * H].T
        c[0, sl, :] = res[i]["cT"][0:H].T
        c[1, sl, :] = res[i]["cT"][H:2 * H].T
    out += np.float32(b_fc[0])
    return out, (h, c)


# revision 18
# speedup vs baseline: 1.1348x; 1.1348x over previous
"""Trainium2 Bass kernel for a 2-layer LSTM (T=1024, B=4096, IN=1, H=16) + linear head.

Strategy: pure data parallel over batch (B=4096 -> 512 per core on 8 cores).
Within a core, the two LSTM layers are fused into a single recurrence with
layer 1 running one timestep behind layer 0, so each step needs ONE gates
matmul (K=33 = [h1(16); h0(16); x(1)], M=128 = all 8 gate blocks of both
layers), one sigmoid + one tanh activation over the PSUM gate rows and four
vector ops for the cell/hidden updates. The per-core batch is further split
into GROUPS independent interleaved chains so the engines overlap across
groups. States stay resident in SBUF; x is staged in SBUF up front; the fc
head is a [16]-dot applied on host to the exported y1 (h1 history).

The gates matmul runs as float32r (1 cycle/row for N>=256, vs 4 for fp32).
"""
import numpy as np

import concourse.bass as bass
import concourse.bacc as bacc
import concourse.tile as tile
from concourse import mybir
from concourse.bass_utils import run_bass_kernel_spmd

T, B, H, NCORES = 1024, 4096, 16, 8
BL = B // NCORES  # 512 batch elements per core
K = 2 * H + 1     # 33 rows: h1, h0, x
M = 8 * H         # 128 gate rows
GROUPS = 2        # independent interleaved chains per core

F32 = mybir.dt.float32
F32R = mybir.dt.float32r
AF = mybir.ActivationFunctionType

# PyTorch gate order inside 4H weight matrices: i, f, g, o
_GS = {'i': slice(0, H), 'f': slice(H, 2 * H), 'g': slice(2 * H, 3 * H), 'o': slice(3 * H, 4 * H)}
# Output (M) column order of the combined gates matmul: 16 cols per entry.
# Layer 1 first in each pair so the step-0 pipeline-bubble memsets target
# partition base 0 (compute ops may only start at partitions {0,32,64,96}).
# sigmoid rows 0:96 = [i1,i0,f1,f0,o1,o0], tanh rows 96:128 = [g1,g0]
_COL_ORDER = [('i', 1), ('i', 0), ('f', 1), ('f', 0), ('o', 1), ('o', 0), ('g', 1), ('g', 0)]


def _pack_weights(W_ih0, W_hh0, b_ih0, b_hh0, W_ih1, W_hh1, b_ih1, b_hh1):
    Wcomb = np.zeros((K, M), np.float32)
    bcomb = np.zeros((M, 1), np.float32)
    for ci, (gate, layer) in enumerate(_COL_ORDER):
        cols = slice(ci * H, (ci + 1) * H)
        gs = _GS[gate]
        if layer == 0:
            # state rows: 0:16 h1, 16:32 h0, 32 x
            Wcomb[2 * H, cols] = W_ih0[gs, 0]
            Wcomb[H:2 * H, cols] = W_hh0[gs, :].T
            bcomb[cols, 0] = b_ih0[gs] + b_hh0[gs]
        else:
            Wcomb[H:2 * H, cols] = W_ih1[gs, :].T
            Wcomb[0:H, cols] = W_hh1[gs, :].T
            bcomb[cols, 0] = b_ih1[gs] + b_hh1[gs]
    return Wcomb, bcomb


def _build(t_steps=T, bl=BL, groups=GROUPS):
    gb = bl // groups  # batch per group
    nc = bacc.Bacc()
    x_d = nc.declare_dram_parameter("x", [t_steps, bl], F32, isOutput=False)
    w_d = nc.declare_dram_parameter("wcomb", [K, M], F32, isOutput=False)
    b_d = nc.declare_dram_parameter("bcomb", [M, 1], F32, isOutput=False)
    y1_d = nc.declare_dram_parameter("y1", [t_steps, H, bl], F32, isOutput=True)
    hT_d = nc.declare_dram_parameter("hT", [2 * H, bl], F32, isOutput=True)
    cT_d = nc.declare_dram_parameter("cT", [2 * H, bl], F32, isOutput=True)

    P = min(t_steps, 128)           # x staging partitions
    NCH = (t_steps + P - 1) // P    # x staging chunks
    assert P * NCH == t_steps

    with tile.TileContext(nc) as tc:
        with (
            tc.tile_pool(name="singles", bufs=1) as singles,
            tc.tile_pool(name="psg", bufs=3, space="PSUM") as psg_pool,
            tc.tile_pool(name="acts", bufs=3) as acts_pool,
            tc.tile_pool(name="cells", bufs=3) as cells_pool,
        ):
            w_sb = singles.tile([K, M], F32)
            nc.sync.dma_start(out=w_sb[:, :].bitcast(F32R), in_=w_d[:, :].bitcast(F32R))
            b_sb = singles.tile([M, 1], F32)
            nc.sync.dma_start(out=b_sb[:, :], in_=b_d[:, :])

            x_all = singles.tile([P, NCH, bl], F32)
            nc.sync.dma_start(out=x_all[:, :, :], in_=x_d.rearrange("(n p) b -> p n b", p=P))

            zb = singles.tile([4 * H, 1], F32)
            nc.vector.memset(zb[:, :], 0.0)

            # Per-group persistent state: rows 0:16 h1, 16:32 h0, row 32 x.
            # c lives at rows 32:64 of a 64-row tile so every TensorTensor's
            # two inputs share a start partition (walrus requires equal SBUF
            # input start partitions, and starts must be in {0,32,64,96}).
            st = [[singles.tile([K, gb], F32, name=f"state{g}_{i}") for i in range(2)]
                  for g in range(groups)]
            cs = [[singles.tile([4 * H, gb], F32, name=f"cell{g}_{i}") for i in range(2)]
                  for g in range(groups)]
            for g in range(groups):
                gsl = slice(g * gb, (g + 1) * gb)
                nc.sync.dma_start(out=st[g][0][2 * H:2 * H + 1, :].bitcast(F32R),
                                  in_=x_all[0:1, 0, gsl].bitcast(F32R))
                nc.vector.memset(st[g][0][0:2 * H, :], 0.0)
                nc.vector.memset(cs[g][0][2 * H:4 * H, :], 0.0)

            for t in range(t_steps + 1):
                sigs, gts, psums = [], [], []
                for g in range(groups):
                    S = st[g][t % 2]
                    psum_g = psg_pool.tile([M, gb], F32, tag=f"psg{g}", name=f"psg{g}")
                    nc.tensor.matmul(psum_g[:, :], w_sb[:, :].bitcast(F32R),
                                     S[:, :].bitcast(F32R), start=True, stop=True)
                    sig = acts_pool.tile([6 * H, gb], F32, tag=f"sig{g}", name=f"sig{g}")
                    nc.scalar.activation(sig[:, :], psum_g[0:6 * H, :], AF.Sigmoid,
                                         bias=b_sb[0:6 * H, :], scale=1.0)
                    gt = acts_pool.tile([2 * H, gb], F32, tag=f"gt{g}", name=f"gt{g}")
                    nc.scalar.activation(gt[:, :], psum_g[6 * H:8 * H, :], AF.Tanh,
                                         bias=b_sb[6 * H:8 * H, :], scale=1.0)
                    sigs.append(sig)
                    gts.append(gt)

                for g in range(groups):
                    sig, gt = sigs[g], gts[g]
                    cc = cs[g][t % 2]
                    cn = cs[g][(t + 1) % 2]
                    fcm = cells_pool.tile([4 * H, gb], F32, tag=f"fcm{g}", name=f"fcm{g}")
                    nc.vector.tensor_mul(fcm[2 * H:4 * H, :], sig[2 * H:4 * H, :], cc[2 * H:4 * H, :])
                    mt = cells_pool.tile([4 * H, gb], F32, tag=f"mt{g}", name=f"mt{g}")
                    nc.vector.tensor_mul(mt[2 * H:4 * H, :], sig[0:2 * H, :], gt[:, :])
                    nc.vector.tensor_add(cn[2 * H:4 * H, :], fcm[2 * H:4 * H, :], mt[2 * H:4 * H, :])

                for g in range(groups):
                    gsl = slice(g * gb, (g + 1) * gb)
                    sig = sigs[g]
                    Sn = st[g][(t + 1) % 2]
                    cn = cs[g][(t + 1) % 2]
                    tcl = cells_pool.tile([6 * H, gb], F32, tag=f"tcl{g}", name=f"tcl{g}")
                    nc.scalar.activation(tcl[4 * H:6 * H, :], cn[2 * H:4 * H, :], AF.Tanh,
                                         bias=zb[2 * H:4 * H, :])
                    nc.vector.tensor_mul(Sn[0:2 * H, :].bitcast(F32R), sig[4 * H:6 * H, :], tcl[4 * H:6 * H, :])

                    if t == 0:
                        # layer1 pipeline bubble: zero the garbage h1/c1 of step 0
                        nc.vector.memset(Sn[0:H, :], 0.0)
                        nc.vector.memset(cn[2 * H:3 * H, :], 0.0)

                    if t >= 1:
                        # export h1_{t-1} (rows 0:16 of Sn); fc head applied on host
                        nc.sync.dma_start(out=y1_d[t - 1, :, gsl], in_=Sn[0:H, :])

                    # prefetch x_{t+1} into the next state tile's x row
                    if t + 1 <= t_steps - 1:
                        tn = t + 1
                        nc.sync.dma_start(out=Sn[2 * H:2 * H + 1, :].bitcast(F32R),
                                          in_=x_all[tn % P:tn % P + 1, tn // P, gsl].bitcast(F32R))

            # final hidden states (hT_d rows 0:16 = h0, 16:32 = h1):
            #   h0_{T-1} lives in st[.][T%2] rows 16:32 (written at step T-1)
            #   h1_{T-1} lives in st[.][(T+1)%2] rows 0:16 (written at step T)
            for g in range(groups):
                gsl = slice(g * gb, (g + 1) * gb)
                nc.sync.dma_start(out=hT_d[0:H, gsl], in_=st[g][t_steps % 2][H:2 * H, :])
                nc.sync.dma_start(out=hT_d[H:2 * H, gsl], in_=st[g][(t_steps + 1) % 2][0:H, :])
                nc.sync.dma_start(out=cT_d[0:H, gsl], in_=cs[g][t_steps % 2][3 * H:4 * H, :])
                nc.sync.dma_start(out=cT_d[H:2 * H, gsl], in_=cs[g][(t_steps + 1) % 2][2 * H:3 * H, :])

    nc.compile()
    return nc


_NC_CACHE = {}


def _get_nc(t_steps=T, bl=BL):
    key = (t_steps, bl)
    if key not in _NC_CACHE:
        _NC_CACHE[key] = _build(t_steps, bl)
    return _NC_CACHE[key]


def kernel(x, W_ih0, W_hh0, b_ih0, b_hh0, W_ih1, W_hh1, b_ih1, b_hh1, W_fc, b_fc):
    x = np.asarray(x, np.float32)
    Wcomb, bcomb = _pack_weights(
        np.asarray(W_ih0, np.float32), np.asarray(W_hh0, np.float32),
        np.asarray(b_ih0, np.float32), np.asarray(b_hh0, np.float32),
        np.asarray(W_ih1, np.float32), np.asarray(W_hh1, np.float32),
        np.asarray(b_ih1, np.float32), np.asarray(b_hh1, np.float32))

    nc = _get_nc()
    xs = x[:, :, 0]  # [T, B]
    in_maps = []
    for i in range(NCORES):
        sl = slice(i * BL, (i + 1) * BL)
        in_maps.append({
            "x": np.ascontiguousarray(xs[:, sl]),
            "wcomb": Wcomb, "bcomb": bcomb,
        })
    res = run_bass_kernel_spmd(nc, in_maps, list(range(NCORES))).results

    wfc = np.asarray(W_fc, np.float32)[0]  # [H]
    out = np.empty((T, B, 1), np.float32)
    h = np.empty((2, B, H), np.float32)
    c = np.empty((2, B, H), np.float32)
    for i in range(NCORES):
        sl = slice(i * BL, (i + 1) * BL)
        # y1: [T, H, BL] -> out = sum_h y1*wfc
        out[:, sl, 0] = np.einsum('thb,h->tb', res[i]["y1"], wfc, optimize=True)
        h[0, sl, :] = res[i]["hT"][0:H].T
        h[1, sl, :] = res[i]["hT"][H:2 * H].T
        c[0, sl, :] = res[i]["cT"][0:H].T
        c[1, sl, :] = res[i]["cT"][H:2 * H].T
    out += np.float32(b_fc[0])
    return out, (h, c)
